# revision 6
# baseline (speedup 1.0000x reference)
"""Trainium2 Bass kernel v2.1: 2-layer hetero-GCN + linear edge decoder, bf16.

Per (dst-tile, relation): 640 edge slots = 2 "A" chunks (srcs in x[0:32768]) +
3 "B" chunks (srcs in x[17232:50000]); the flexible band [17232,32768)
balances A to exactly <=256. One-hot scatter matrices s [edge, dst] fold the
edge weight w = ns[src]*nd[dst]; z = s^T g on TensorE in [dst, feat] layout;
zT via ONE batched XBAR dma-transpose per (tile, rel); W-matmuls and the
decoder fold (m = h @ (Wb | Wp1@Wp2)) in bf16; bias rides a constant-1 column
(slot 639, zero here since biases are 0). h PSUM accumulates with a memset +
start=False (two accumulation regions share a PSUM bank; a second start=True
wipes the bank on HW).

Decoder: score[e] = u[src]+v[dst]+c is linear; each core's 50k edges are
bucketed by endpoint>>7 twice (by src -> u part, by dst -> v part); each
128-node bucket keeps <=128 edges expanded by an exact one-hot matmul;
overflow edges (~1.7k/dir) go through a small SWDGE gather of packed uv rows,
extracted host-side. Host un-permutes and adds the parts.
"""
import os
import sys

for _p in ("/opt/trn_rl_repo", "/root/.axon_site/_ro/trn_rl_repo"):
    if os.path.isdir(_p) and _p not in sys.path:
        sys.path.append(_p)

import numpy as np
import ml_dtypes

import concourse.bass as bass
import concourse.bacc as bacc
import concourse.mybir as mybir
import concourse.tile as tile
from concourse.bass_utils import run_bass_kernel_spmd
from concourse.masks import make_identity

P = 128
NC = 8
R = 8
F32 = mybir.dt.float32
BF16 = mybir.dt.bfloat16
I16 = mybir.dt.int16
BF = ml_dtypes.bfloat16

XC = 640           # padded x row (bf16 -> 1280B, mult of 256)
NA, NB = 256, 384  # A/B slots per (tile, rel)
NSLOT = NA + NB
HI_BASE = 17232    # B table = x[17232:50000] (32768 rows)
LO_SIZE = 32768
OVF = 2560         # decoder overflow slots per direction
AFT = mybir.ActivationFunctionType


def _deg_norm(idx, n):
    deg = np.bincount(idx, minlength=n).astype(np.float32)
    out = np.zeros(n, np.float32)
    nz = deg > 0
    out[nz] = 1.0 / np.sqrt(np.maximum(deg[nz], 1.0))
    return out


def _wrap16(a):
    # [S] -> [16, S/16] wrapped, tiled to [128, S/16]
    S = a.shape[0]
    w = a.reshape(S // 16, 16).T
    return np.tile(w, (8, 1))


def _build(dims):
    tpc, n, npad, nbk = dims["tpc"], dims["n"], dims["npad"], dims["nbk"]
    nc = bacc.Bacc("TRN2", target_bir_lowering=False, debug=False)

    xcat = nc.declare_dram_parameter("xcat", [n, XC], BF16, isOutput=False)
    WA = nc.declare_dram_parameter("WA", [P, R * 5 * 256], BF16, isOutput=False)
    WBIAS = nc.declare_dram_parameter("WBIAS", [P, 512], BF16, isOutput=False)
    WM = nc.declare_dram_parameter("WM", [P, 4 * P], BF16, isOutput=False)
    iota_rep = nc.declare_dram_parameter("iota_rep", [P, P], BF16, isOutput=False)
    iota_col = nc.declare_dram_parameter("iota_col", [P, 1], F32, isOutput=False)
    one_col = nc.declare_dram_parameter("one_col", [P, 1], BF16, isOutput=False)
    idxa = nc.declare_dram_parameter("idxa", [P, tpc * 128], I16, isOutput=False)
    idxb = nc.declare_dram_parameter("idxb", [P, tpc * 192], I16, isOutput=False)
    meta_d = nc.declare_dram_parameter("meta_d", [P, tpc * 40], F32, isOutput=False)
    meta_w = nc.declare_dram_parameter("meta_w", [P, tpc * 40], F32, isOutput=False)
    meta_nd = nc.declare_dram_parameter("meta_nd", [P, tpc * 40], F32, isOutput=False)
    meta_nw = nc.declare_dram_parameter("meta_nw", [P, tpc * 40], F32, isOutput=False)
    dloc_u = nc.declare_dram_parameter("dloc_u", [P, nbk * 2], F32, isOutput=False)
    dloc_v = nc.declare_dram_parameter("dloc_v", [P, nbk * 2], F32, isOutput=False)
    su_out = nc.declare_dram_parameter("su_out", [nbk * 256, 8], F32, isOutput=True)
    sv_out = nc.declare_dram_parameter("sv_out", [nbk * 256, 8], F32, isOutput=True)
    DEBUG = bool(os.environ.get("K2_DEBUG"))
    if DEBUG:
        m_dbg = nc.declare_dram_parameter("m_dbg", [tpc * P, P], F32, isOutput=True)
        uv_dbg = nc.declare_dram_parameter("uv_dbg", [tpc * P, 16], F32, isOutput=True)
        h_dbg = nc.declare_dram_parameter("h_dbg", [tpc * P, 512], F32, isOutput=True)

    with tile.TileContext(nc) as tc:
        with (
            tc.tile_pool(name="cpool", bufs=1) as cp,
            tc.tile_pool(name="dram", bufs=1, space="DRAM") as dp,
        ):
            m_shard = dp.tile([tpc * P, P], BF16)
            m_full = dp.tile([npad, P], BF16, addr_space="Shared")
            uv_shard = dp.tile([tpc * P, 16], BF16)
            uv_full = dp.tile([npad, 16], BF16, addr_space="Shared")

            ident = cp.tile([P, P], BF16)
            make_identity(nc, ident[:])
            iota_t = cp.tile([P, P], BF16)
            nc.sync.dma_start(out=iota_t[:], in_=iota_rep[:, :])
            iotac_t = cp.tile([P, 1], F32)
            nc.sync.dma_start(out=iotac_t[:], in_=iota_col[:, :])
            one_t = cp.tile([P, 1], BF16)
            nc.sync.dma_start(out=one_t[:], in_=one_col[:, :])
            wa_t = cp.tile([P, R * 5 * 256], BF16)
            nc.sync.dma_start(out=wa_t[:], in_=WA[:, :])
            wb_t = cp.tile([P, 512], BF16)
            nc.sync.dma_start(out=wb_t[:], in_=WBIAS[:, :])
            wm_t = cp.tile([P, 4 * P], BF16)
            nc.sync.dma_start(out=wm_t[:], in_=WM[:, :])

            def build_s(sp, me_d, me_w, me_nd, me_nw, tag):
                """s [128, 40*128] bf16 for one tile; A chunks on DVE,
                B chunks on Scalar (abs/relu trick)."""
                s = sp.tile([P, 40 * P], BF16, tag=tag)
                a1 = sp.tile([P, P], BF16, tag=tag + "a1")
                for ci in range(40):
                    if ci < 16:
                        nc.vector.tensor_scalar(
                            out=s[:, ci * P:(ci + 1) * P], in0=iota_t[:],
                            scalar1=me_d[:, ci:ci + 1], scalar2=me_w[:, ci:ci + 1],
                            op0=mybir.AluOpType.is_equal, op1=mybir.AluOpType.mult)
                    else:
                        nc.scalar.activation(
                            out=a1[:], in_=iota_t[:], func=AFT.Abs,
                            bias=me_nd[:, ci:ci + 1], scale=1.0)
                        nc.scalar.activation(
                            out=s[:, ci * P:(ci + 1) * P], in_=a1[:], func=AFT.Relu,
                            bias=me_w[:, ci:ci + 1], scale=me_nw[:, ci:ci + 1])
                return s

            # ---------------- layer A ----------------
            with (
                tc.tile_pool(name="gpool", bufs=2) as gp,
                tc.tile_pool(name="spool", bufs=3) as sp,
                tc.tile_pool(name="zpool", bufs=3) as zp,
                tc.tile_pool(name="mpool", bufs=2) as mp,
                tc.tile_pool(name="hpool", bufs=2) as hp,
                tc.tile_pool(name="psz", bufs=2, space="PSUM") as psz,
                tc.tile_pool(name="pst", bufs=2, space="PSUM") as pst,
                tc.tile_pool(name="psh", bufs=1, space="PSUM") as psh,
                tc.tile_pool(name="psm", bufs=1, space="PSUM") as psm,
            ):
                for t in range(tpc):
                    ia_t = mp.tile([P, 128], I16, tag="ia")
                    nc.sync.dma_start(out=ia_t[:], in_=idxa[:, t * 128:(t + 1) * 128])
                    ib_t = mp.tile([P, 192], I16, tag="ib")
                    nc.sync.dma_start(out=ib_t[:], in_=idxb[:, t * 192:(t + 1) * 192])
                    md_t = mp.tile([P, 40], F32, tag="md")
                    nc.sync.dma_start(out=md_t[:], in_=meta_d[:, t * 40:(t + 1) * 40])
                    mw_t = mp.tile([P, 40], F32, tag="mw")
                    nc.sync.dma_start(out=mw_t[:], in_=meta_w[:, t * 40:(t + 1) * 40])
                    mnd_t = mp.tile([P, 40], F32, tag="mnd")
                    nc.sync.dma_start(out=mnd_t[:], in_=meta_nd[:, t * 40:(t + 1) * 40])
                    mnw_t = mp.tile([P, 40], F32, tag="mnw")
                    nc.sync.dma_start(out=mnw_t[:], in_=meta_nw[:, t * 40:(t + 1) * 40])

                    g = gp.tile([P, 40, XC], BF16, tag="gA")
                    for q in range(2):
                        nc.gpsimd.dma_gather(
                            out_ap=g[:, 8 * q:8 * (q + 1), :], in_ap=xcat[0:LO_SIZE, :],
                            idxs_ap=ia_t[:, 64 * q:64 * (q + 1)],
                            num_idxs=1024, num_idxs_reg=1024, elem_size=XC)
                    for q in range(3):
                        nc.gpsimd.dma_gather(
                            out_ap=g[:, 16 + 8 * q:24 + 8 * q, :],
                            in_ap=xcat[HI_BASE:n, :],
                            idxs_ap=ib_t[:, 64 * q:64 * (q + 1)],
                            num_idxs=1024, num_idxs_reg=1024, elem_size=XC)

                    s = build_s(sp, md_t, mw_t, mnd_t, mnw_t, "sA")

                    h_ps = psh.tile([P, 512], F32, tag="hps")
                    nc.vector.memset(h_ps[:], 0.0)
                    for r in range(R):
                        cis = [2 * r, 2 * r + 1, 16 + 3 * r, 17 + 3 * r, 18 + 3 * r]
                        z_ps = psz.tile([P, XC], F32, tag="zps")
                        for k, ci in enumerate(cis):
                            nc.tensor.matmul(
                                z_ps[:, 0:512], s[:, ci * P:(ci + 1) * P],
                                g[:, ci, 0:512],
                                start=(k == 0), stop=(k == 4))
                            nc.tensor.matmul(
                                z_ps[:, 512:XC], s[:, ci * P:(ci + 1) * P],
                                g[:, ci, 512:XC],
                                start=(k == 0), stop=(k == 4))
                        zs = zp.tile([P, XC], BF16, tag="zs")
                        nc.scalar.copy(out=zs[:, 0:512], in_=z_ps[:, 0:512])
                        nc.scalar.copy(out=zs[:, 512:XC], in_=z_ps[:, 512:XC])
                        nc.vector.tensor_copy(out=zs[:, 639:640], in_=one_t[:])
                        zT = zp.tile([P, 5 * P], BF16, tag="zT")
                        for f in range(5):
                            ztp = pst.tile([P, P], BF16, tag="ztp")
                            nc.tensor.transpose(out=ztp[:], in_=zs[:, f * P:(f + 1) * P],
                                                identity=ident[:])
                            nc.vector.tensor_copy(out=zT[:, f * P:(f + 1) * P],
                                                  in_=ztp[:])
                        for f in range(2):   # h2 <- x2 chunks
                            nc.tensor.matmul(
                                h_ps[:, 0:256], zT[:, f * P:(f + 1) * P],
                                wa_t[:, (r * 5 + f) * 256:(r * 5 + f + 1) * 256],
                                start=False, stop=False)
                        for f in range(2, 5):  # h3 <- x3 chunks
                            nc.tensor.matmul(
                                h_ps[:, 256:512], zT[:, f * P:(f + 1) * P],
                                wa_t[:, (r * 5 + f) * 256:(r * 5 + f + 1) * 256],
                                start=False, stop=False)
                        if r == R - 1:
                            nc.tensor.matmul(h_ps[:, 0:256], zT[:, 4 * P:5 * P],
                                             wb_t[:, 0:256], start=False, stop=True)
                            nc.tensor.matmul(h_ps[:, 256:512], zT[:, 4 * P:5 * P],
                                             wb_t[:, 256:512], start=False, stop=True)
                    hsb = hp.tile([P, 512], BF16, tag="hsb")
                    nc.scalar.activation(out=hsb[:], in_=h_ps[:], func=AFT.Relu)
                    m_ps = psm.tile([P, P], F32, tag="mps")
                    for j in range(4):
                        htp = pst.tile([P, P], BF16, tag="ztp")
                        nc.tensor.transpose(out=htp[:], in_=hsb[:, j * P:(j + 1) * P],
                                            identity=ident[:])
                        hts = zp.tile([P, P], BF16, tag="hts")
                        nc.vector.tensor_copy(out=hts[:], in_=htp[:])
                        nc.tensor.matmul(m_ps[:], hts[:],
                                         wm_t[:, j * P:(j + 1) * P],
                                         start=(j == 0), stop=(j == 3))
                    msb = hp.tile([P, P], BF16, tag="msb")
                    nc.scalar.copy(out=msb[:], in_=m_ps[:])
                    nc.sync.dma_start(out=m_shard[t * P:(t + 1) * P, :], in_=msb[:])

            nc.gpsimd.collective_compute(
                "AllGather", mybir.AluOpType.bypass,
                replica_groups=[list(range(NC))],
                ins=[m_shard[:, :]], outs=[m_full[:, :]])

            # ---------------- layer B (m-space aggregation) ----------------
            with (
                tc.tile_pool(name="gpoolb", bufs=2) as gp,
                tc.tile_pool(name="spoolb", bufs=3) as sp,
                tc.tile_pool(name="mpoolb", bufs=2) as mp,
                tc.tile_pool(name="uvp", bufs=2) as uvp,
                tc.tile_pool(name="psu", bufs=2, space="PSUM") as psu,
            ):
                for t in range(tpc):
                    ia_t = mp.tile([P, 128], I16, tag="iab")
                    nc.sync.dma_start(out=ia_t[:], in_=idxa[:, t * 128:(t + 1) * 128])
                    ib_t = mp.tile([P, 192], I16, tag="ibb")
                    nc.sync.dma_start(out=ib_t[:], in_=idxb[:, t * 192:(t + 1) * 192])
                    md_t = mp.tile([P, 40], F32, tag="mdb")
                    nc.sync.dma_start(out=md_t[:], in_=meta_d[:, t * 40:(t + 1) * 40])
                    mw_t = mp.tile([P, 40], F32, tag="mwb")
                    nc.sync.dma_start(out=mw_t[:], in_=meta_w[:, t * 40:(t + 1) * 40])
                    mnd_t = mp.tile([P, 40], F32, tag="mndb")
                    nc.sync.dma_start(out=mnd_t[:], in_=meta_nd[:, t * 40:(t + 1) * 40])
                    mnw_t = mp.tile([P, 40], F32, tag="mnwb")
                    nc.sync.dma_start(out=mnw_t[:], in_=meta_nw[:, t * 40:(t + 1) * 40])

                    g = gp.tile([P, 40, P], BF16, tag="gB")
                    for q in range(2):
                        nc.gpsimd.dma_gather(
                            out_ap=g[:, 8 * q:8 * (q + 1), :],
                            in_ap=m_full[0:LO_SIZE, :],
                            idxs_ap=ia_t[:, 64 * q:64 * (q + 1)],
                            num_idxs=1024, num_idxs_reg=1024, elem_size=P)
                    for q in range(3):
                        nc.gpsimd.dma_gather(
                            out_ap=g[:, 16 + 8 * q:24 + 8 * q, :],
                            in_ap=m_full[HI_BASE:n, :],
                            idxs_ap=ib_t[:, 64 * q:64 * (q + 1)],
                            num_idxs=1024, num_idxs_reg=1024, elem_size=P)

                    s = build_s(sp, md_t, mw_t, mnd_t, mnw_t, "sB")

                    uv_ps = psu.tile([P, 16], F32, tag="uvps")
                    for r in range(R):
                        cis = [2 * r, 2 * r + 1, 16 + 3 * r, 17 + 3 * r, 18 + 3 * r]
                        for k, ci in enumerate(cis):
                            nc.tensor.matmul(
                                uv_ps[:], s[:, ci * P:(ci + 1) * P],
                                g[:, ci, r * 16:(r + 1) * 16],
                                start=(r == 0 and k == 0),
                                stop=(r == R - 1 and k == 4))
                    uvsb = uvp.tile([P, 16], BF16, tag="uvsb")
                    nc.scalar.copy(out=uvsb[:], in_=uv_ps[:])
                    nc.sync.dma_start(out=uv_shard[t * P:(t + 1) * P, :], in_=uvsb[:])

            nc.gpsimd.collective_compute(
                "AllGather", mybir.AluOpType.bypass,
                replica_groups=[list(range(NC))],
                ins=[uv_shard[:, :]], outs=[uv_full[:, :]])

            # ---------------- decoder (bucketed one-hot expansion) -------
            with (
                tc.tile_pool(name="dwp", bufs=3) as wp_,
                tc.tile_pool(name="dsp", bufs=3) as dsp,
                tc.tile_pool(name="dlp", bufs=2) as dlp,
                tc.tile_pool(name="dop", bufs=3) as dop,
                tc.tile_pool(name="psd", bufs=4, space="PSUM") as psd,
            ):
                suv = su_out.ap().rearrange("(b c p) d -> b p c d", c=2, p=P)
                svv = sv_out.ap().rearrange("(b c p) d -> b p c d", c=2, p=P)
                BK = 16  # buckets per dloc load
                for b in range(nbk):
                    if b % BK == 0:
                        bh = min(BK, nbk - b)
                        lu_t = dlp.tile([P, BK * 2], F32, tag="lu")
                        nc.sync.dma_start(
                            out=lu_t[:, 0:bh * 2],
                            in_=dloc_u[:, b * 2:(b + bh) * 2])
                        lv_t = dlp.tile([P, BK * 2], F32, tag="lv")
                        nc.sync.dma_start(
                            out=lv_t[:, 0:bh * 2],
                            in_=dloc_v[:, b * 2:(b + bh) * 2])
                    win = wp_.tile([P, 16], BF16, tag="win")
                    nc.sync.dma_start(out=win[:], in_=uv_full[b * P:(b + 1) * P, :])
                    o_u = dop.tile([P, 2, 8], F32, tag="ou")
                    o_v = dop.tile([P, 2, 8], F32, tag="ov")
                    for (loc, cols, ot) in ((lu_t, (0, 8), o_u), (lv_t, (8, 16), o_v)):
                        for c in range(2):
                            off = (b % BK) * 2 + c
                            sdt = dsp.tile([P, P], BF16, tag="dsdt")
                            nc.vector.tensor_scalar(
                                out=sdt[:], in0=iota_t[:],
                                scalar1=loc[:, off:off + 1], scalar2=None,
                                op0=mybir.AluOpType.is_equal)
                            sd_ps = psd.tile([P, P], BF16, tag="sdps")
                            nc.tensor.transpose(out=sd_ps[:], in_=sdt[:],
                                                identity=ident[:])
                            sd = dsp.tile([P, P], BF16, tag="dsd")
                            nc.vector.tensor_copy(out=sd[:], in_=sd_ps[:])
                            sc_ps = psd.tile([P, 8], F32, tag="scps")
                            nc.tensor.matmul(sc_ps[:], sd[:], win[:, cols[0]:cols[1]],
                                             start=True, stop=True)
                            nc.scalar.copy(out=ot[:, c, :], in_=sc_ps[:])
                    nc.sync.dma_start(out=suv[b], in_=o_u[:])
                    nc.sync.dma_start(out=svv[b], in_=o_v[:])
    nc.finalize()
    return nc


def _prep(inputs):
    x2 = np.asarray(inputs["node2_features"], np.float32)
    x3 = np.asarray(inputs["mpnn_features"], np.float32)
    src = np.asarray(inputs["src"])
    dst = np.asarray(inputs["dst"])
    dec_src = np.asarray(inputs["dec_src"]).astype(np.int64)
    dec_dst = np.asarray(inputs["dec_dst"]).astype(np.int64)
    W2a = np.asarray(inputs["W2a"], np.float32)
    b2a = np.asarray(inputs["b2a"], np.float32)
    W2b = np.asarray(inputs["W2b"], np.float32)
    b2b = np.asarray(inputs["b2b"], np.float32)
    W3a = np.asarray(inputs["W3a"], np.float32)
    b3a = np.asarray(inputs["b3a"], np.float32)
    W3b = np.asarray(inputs["W3b"], np.float32)
    b3b = np.asarray(inputs["b3b"], np.float32)
    Wp1 = np.asarray(inputs["Wp1"], np.float32)
    bp1 = np.asarray(inputs["bp1"], np.float32)
    Wp2 = np.asarray(inputs["Wp2"], np.float32)
    bp2 = np.asarray(inputs["bp2"], np.float32)

    n = x2.shape[0]
    Rr, E = src.shape
    assert Rr == R and n == 50000
    ed = dec_src.shape[0]
    d2, d3 = x2.shape[1], x3.shape[1]
    assert d2 == 256 and d2 + d3 <= XC - 1
    ntiles = NC * (-(-n // (P * NC)))
    tpc = ntiles // NC
    npad = ntiles * P

    # ---- per-edge weights ----
    ns_arr = np.stack([_deg_norm(src[r], n) for r in range(R)])
    nd_arr = np.stack([_deg_norm(dst[r], n) for r in range(R)])
    src_f = src.astype(np.int64).ravel()
    dst_f = dst.astype(np.int64).ravel()
    rel_f = np.repeat(np.arange(R, dtype=np.int64), E)
    w_f = (ns_arr[rel_f, src_f] * nd_arr[rel_f, dst_f]).astype(np.float32)

    tile_f = dst_f >> 7
    key = tile_f * R + rel_f
    ngroup = ntiles * R
    counts = np.bincount(key, minlength=ngroup)
    assert counts.max() <= NSLOT, counts.max()

    band = np.where(src_f < HI_BASE, 0, np.where(src_f < LO_SIZE, 1, 2))
    order = np.lexsort((band, key))
    so, do_, wo, ko = src_f[order], dst_f[order], w_f[order], key[order]
    starts = np.zeros(ngroup, np.int64)
    np.cumsum(counts[:-1], out=starts[1:])
    rank = np.arange(len(so)) - starts[ko]
    mustA = np.bincount(key[band == 0], minlength=ngroup)
    mustB_start = np.bincount(key[band < 2], minlength=ngroup)
    assert mustA.max() <= NA
    nA = np.minimum(NA, mustB_start)
    nB = counts - nA
    assert nB.max() <= NB, nB.max()
    in_A = rank < nA[ko]
    slotA = rank
    slotB = rank - nA[ko]

    idxA = np.zeros((ngroup, NA), np.int16)
    idxB = np.zeros((ngroup, NB), np.int16)
    dst_sl = np.zeros((ngroup, NSLOT), np.float32)
    w_sl = np.zeros((ngroup, NSLOT), np.float32)
    idxA[ko[in_A], slotA[in_A]] = so[in_A].astype(np.int16)
    idxB[ko[~in_A], slotB[~in_A]] = (so[~in_A] - HI_BASE).astype(np.int16)
    slot_all = np.where(in_A, slotA, NA + slotB)
    dst_sl[ko, slot_all] = (do_ & 127).astype(np.float32)
    w_sl[ko, slot_all] = wo

    gpc = tpc * R
    idxa_c, idxb_c, md_c, mw_c, mnd_c, mnw_c = [], [], [], [], [], []
    for c in range(NC):
        sl = slice(c * gpc, (c + 1) * gpc)
        ia = idxA[sl].reshape(tpc, R * NA)
        ib = idxB[sl].reshape(tpc, R * NB)
        ia_w = np.concatenate([_wrap16(ia[t]) for t in range(tpc)], axis=1)
        ib_w = np.concatenate([_wrap16(ib[t]) for t in range(tpc)], axis=1)
        dsl = dst_sl[sl].reshape(tpc, R, NSLOT)
        wsl = w_sl[sl].reshape(tpc, R, NSLOT)
        d40 = np.zeros((tpc, 40, P), np.float32)
        w40 = np.zeros((tpc, 40, P), np.float32)
        for r in range(R):
            for j in range(2):
                d40[:, 2 * r + j] = dsl[:, r, j * P:(j + 1) * P]
                w40[:, 2 * r + j] = wsl[:, r, j * P:(j + 1) * P]
            for j in range(3):
                d40[:, 16 + 3 * r + j] = dsl[:, r, NA + j * P:NA + (j + 1) * P]
                w40[:, 16 + 3 * r + j] = wsl[:, r, NA + j * P:NA + (j + 1) * P]
        idxa_c.append(np.ascontiguousarray(ia_w))
        idxb_c.append(np.ascontiguousarray(ib_w))
        md_c.append(np.ascontiguousarray(d40.transpose(2, 0, 1).reshape(P, tpc * 40)))
        mw_c.append(np.ascontiguousarray(w40.transpose(2, 0, 1).reshape(P, tpc * 40)))
        mnd_c.append(np.ascontiguousarray((-d40).transpose(2, 0, 1).reshape(P, tpc * 40)))
        mnw_c.append(np.ascontiguousarray((-w40).transpose(2, 0, 1).reshape(P, tpc * 40)))

    # ---- tables ----
    x_cat = np.zeros((n, XC), BF)
    x_cat[:, :d2] = x2.astype(BF)
    x_cat[:, d2:d2 + d3] = x3.astype(BF)

    WAp = np.zeros((R, 5, P, 256), np.float32)
    for r in range(R):
        WAp[r, 0] = W2a[r, 0:128]
        WAp[r, 1] = W2a[r, 128:256]
        WAp[r, 2] = W3a[r, 0:128]
        WAp[r, 3] = W3a[r, 128:256]
        WAp[r, 4, 0:d3 - 256] = W3a[r, 256:d3]
        WAp[r, 4, 127] = 0.0
    WAh = np.ascontiguousarray(
        WAp.transpose(2, 0, 1, 3).reshape(P, R * 5 * 256)).astype(BF)
    WBIASp = np.zeros((P, 512), np.float32)
    WBIASp[127, 0:256] = b2a.sum(0)
    WBIASp[127, 256:512] = b3a.sum(0)
    WBIASh = WBIASp.astype(BF)

    M = Wp1 @ Wp2
    A2, A3, B2, B3 = M[0:128], M[128:256], M[256:384], M[384:512]
    WMcat = np.zeros((512, P), np.float32)
    for r in range(R):
        WMcat[0:256, r * 16:(r + 1) * 16] = W2b[r] @ np.concatenate([A2, B2], axis=1)
        WMcat[256:512, r * 16:(r + 1) * 16] = W3b[r] @ np.concatenate([A3, B3], axis=1)
    WMh = np.ascontiguousarray(
        WMcat.reshape(4, P, P).transpose(1, 0, 2).reshape(P, 4 * P)).astype(BF)
    c_total = (b2b.sum(0) @ np.concatenate([A2, B2], axis=1)
               + b3b.sum(0) @ np.concatenate([A3, B3], axis=1))
    c_total = (c_total[0:8] + c_total[8:16] + bp1 @ Wp2 + bp2).astype(np.float32)

    iota_rep = np.tile(np.arange(P, dtype=np.float32), (P, 1)).astype(BF)
    iota_col = np.arange(P, dtype=np.float32).reshape(P, 1)
    one_col = np.ones((P, 1), np.float32).astype(BF)

    # ---- decoder buckets ----
    nbk = npad // P
    epc = -(-ed // NC)
    du_c, dv_c, slotu_c, slotv_c = [], [], [], []
    for c in range(NC):
        e0 = c * epc
        seg = slice(e0, min(e0 + epc, ed))
        locs, slots = [], []
        for ends in (dec_src[seg], dec_dst[seg]):
            bkt = ends >> 7
            order2 = np.argsort(bkt, kind="stable")
            cnt = np.bincount(bkt, minlength=nbk)
            assert cnt.max() <= 256, cnt.max()
            st = np.zeros(nbk, np.int64)
            np.cumsum(cnt[:-1], out=st[1:])
            rank2 = np.arange(len(ends)) - st[bkt[order2]]
            slot = bkt[order2] * 256 + rank2
            loc = np.full(nbk * 2 * P, 200.0, np.float32)
            loc[slot] = (ends[order2] & 127).astype(np.float32)
            inv_slot = np.empty(len(ends), np.int64)
            inv_slot[order2] = slot
            locs.append(loc)
            slots.append(inv_slot)
        du_c.append(np.ascontiguousarray(locs[0].reshape(nbk * 2, P).T))
        dv_c.append(np.ascontiguousarray(locs[1].reshape(nbk * 2, P).T))
        slotu_c.append(slots[0])
        slotv_c.append(slots[1])

    in_maps = []
    for c in range(NC):
        in_maps.append(dict(
            xcat=x_cat, WA=WAh, WBIAS=WBIASh, WM=WMh,
            iota_rep=iota_rep, iota_col=iota_col, one_col=one_col,
            idxa=idxa_c[c], idxb=idxb_c[c],
            meta_d=md_c[c], meta_w=mw_c[c], meta_nd=mnd_c[c], meta_nw=mnw_c[c],
            dloc_u=du_c[c], dloc_v=dv_c[c],
        ))
    dims = dict(n=n, tpc=tpc, npad=npad, nbk=nbk, epc=epc, ed=ed)
    aux = dict(slotu=slotu_c, slotv=slotv_c, c_total=c_total)
    return in_maps, dims, aux


def _assemble(res, dims, aux):
    epc, ed = dims["epc"], dims["ed"]
    outs = []
    for c in range(NC):
        ln = min(epc, ed - c * epc)
        sc = (np.asarray(res.results[c]["su_out"], np.float32)[aux["slotu"][c][:ln]]
              + np.asarray(res.results[c]["sv_out"], np.float32)[aux["slotv"][c][:ln]]
              + aux["c_total"])
        outs.append(sc)
    return np.concatenate(outs, axis=0).astype(np.float32)


_CACHE = {}


def kernel(**inputs):
    in_maps, dims, aux = _prep(inputs)
    key = (dims["n"], dims["tpc"], dims["nbk"])
    nc = _CACHE.get(key)
    if nc is None:
        nc = _build(dims)
        _CACHE[key] = nc
    res = run_bass_kernel_spmd(nc, in_maps, list(range(NC)))
    return np.ascontiguousarray(_assemble(res, dims, aux))


if __name__ == "__main__":
    pass
